# revision 14
# baseline (speedup 1.0000x reference)
"""Per-core causal self-attention kernel (Bass/Tile, TRN2), v2.

One core handles one batch b and one head-group of HL=8 heads (q=k=v quirk:
only the k-slice of W_attn matters):
    K  = x @ Wk + bk                    # [T, NW], NW = HL*64
    per head h: S = K_h K_h^T / 8 (causal), P = exp (no max-sub)
    Y_h = (P @ K_h) / rowsum(P)
    out_partial = Y @ Wp                # [T, COUT]; host sums partials + b_proj

v2 changes vs v1:
  * kt-gen via fp8e4 DoubleRow matmuls with residual correction:
    K ~= (x8 @ w8 + x8 @ wr + xr @ w8)/WSCALE, w quantized at x64 scale.
    12 DR steps x 256 out-cycles vs 8 fp16 steps x 512 (25% less PE).
  * U computed in NATURAL layout: stationary = E^T 128x128 block, moving =
    [V_h | ones] (65 cols) -> out[i, d] accumulated over j-blocks in PSUM.
    65-col moving vs 512-col strips: halves the U matmul PE time. The ones
    column yields the softmax denominator in col 64.
  * normalize: strided reciprocal over the 4 denominator columns + one
    broadcast tensor_tensor mul straight out of PSUM -> yn natural chunk.
  * Y^T for proj recovered by PE transposes of yn (pairs two heads to use
    all 128 partitions), landing in the same y layout v1 used for proj.
  * exp split between ACT (scalar.activation Exp, scale=1/8) and DVE
    (tensor_tensor pow: base e^(1/8) f32 tile ** S) to unbottleneck the
    ACT engine (GPSIMD cannot read PSUM, so Pool is out for exp).
  * causal masks on Pool (SBUF-only op), proj result DMA'd straight from
    PSUM to DRAM (f32) so no DVE copy.
  * S strips of chunk ci interleaved with U chains of chunk ci-1 so PE has
    work while ACT/Pool chew on the exps.
  * out stored fp16, host accumulates in f32.
"""

from contextlib import ExitStack

import concourse.bass as bass
import concourse.tile as tile
from concourse import mybir

F32 = mybir.dt.float32
F16 = mybir.dt.float16
BF16 = mybir.dt.bfloat16
E4M3 = mybir.dt.float8e4
EXP = mybir.ActivationFunctionType.Exp
DR = mybir.MatmulPerfMode.DoubleRow
MUL = mybir.AluOpType.mult
ADD = mybir.AluOpType.add
POW = mybir.AluOpType.pow

WSCALE = 64.0  # power-of-two pre-scale of Wk so fp8e4 residuals stay normal


class Cfg:
    def __init__(self, T=2048, CIN=1024, HL=8, COUT=1024):
        self.T, self.CIN, self.HL, self.COUT = T, CIN, HL, COUT
        assert HL % 2 == 0 and T % 512 == 0 and CIN % 256 == 0 and COUT % 512 == 0
        self.NW = HL * 64          # n width (local head dims)
        self.NB = self.NW // 128   # head-pair blocks
        self.TB = T // 128         # t blocks
        self.NCH = T // 512        # i chunks
        self.CB2 = CIN // 256      # fp8 DoubleRow c-pair chunks


def declare_io(nc, cfg):
    io = {}
    # fp8 DoubleRow operands, host-packed as [CB2, 128, 2, free]
    io["x8"] = nc.dram_tensor("x8", [cfg.CB2, 128, 2, cfg.T], E4M3, kind="ExternalInput")
    io["xr"] = nc.dram_tensor("xr", [cfg.CB2, 128, 2, cfg.T], E4M3, kind="ExternalInput")
    io["w8"] = nc.dram_tensor("w8", [cfg.CB2, 128, 2, cfg.NW], E4M3, kind="ExternalInput")
    io["wr"] = nc.dram_tensor("wr", [cfg.CB2, 128, 2, cfg.NW], E4M3, kind="ExternalInput")
    io["bk"] = nc.dram_tensor("bk", [cfg.NW, 1], F32, kind="ExternalInput")
    io["wp"] = nc.dram_tensor("wp", [cfg.NW, cfg.COUT], F16, kind="ExternalInput")
    io["tri"] = nc.dram_tensor("tri", [128, 128], F16, kind="ExternalInput")
    io["ident"] = nc.dram_tensor("ident", [128, 128], F16, kind="ExternalInput")
    io["out"] = nc.dram_tensor("out", [cfg.T, cfg.COUT], F16, kind="ExternalOutput")
    return io


def build(ctx: ExitStack, tc: tile.TileContext, io, cfg: Cfg,
          pool_exp_stride=0, desc_last=True):
    nc = tc.nc
    T, HL, NB, TB, NCH, CB2, COUT = (cfg.T, cfg.HL, cfg.NB, cfg.TB, cfg.NCH,
                                     cfg.CB2, cfg.COUT)

    consts = ctx.enter_context(tc.tile_pool(name="consts", bufs=1))
    # PSUM budget (8 banks): sps 2x[128,1024]=4, u (ue+uo)x1=2, kps 2x[128,512]=2
    spsum = ctx.enter_context(tc.tile_pool(name="sps", bufs=2, space="PSUM"))
    upsum = ctx.enter_context(tc.tile_pool(name="ups", bufs=1, space="PSUM"))
    kpsum = ctx.enter_context(tc.tile_pool(name="kps", bufs=2, space="PSUM"))
    epool = ctx.enter_context(tc.tile_pool(name="e", bufs=30))
    ynpool = ctx.enter_context(tc.tile_pool(name="yn", bufs=3))
    rpool = ctx.enter_context(tc.tile_pool(name="r", bufs=4))
    opool = ctx.enter_context(tc.tile_pool(name="o", bufs=3))

    # ---- persistent SBUF tensors ----
    tri_t = consts.tile([128, 128], F16, tag="tri")
    nc.sync.dma_start(tri_t[:], io["tri"].ap())
    id_t = consts.tile([128, 128], F16, tag="ident")
    nc.sync.dma_start(id_t[:], io["ident"].ap())
    base_t = consts.tile([128, 1024], F32, tag="base")
    nc.gpsimd.memset(base_t[:], 1.1331484530668263)  # e**(1/8)

    x8_t, xr_t, w8_t, wr_t = [], [], [], []
    for cc in range(CB2):
        t = consts.tile([128, 2, cfg.NW], E4M3, tag=f"w8{cc}", name=f"w8{cc}")
        nc.sync.dma_start(t[:], io["w8"].ap()[cc])
        w8_t.append(t)
        t = consts.tile([128, 2, cfg.NW], E4M3, tag=f"wr{cc}", name=f"wr{cc}")
        nc.sync.dma_start(t[:], io["wr"].ap()[cc])
        wr_t.append(t)
    for cc in range(CB2):
        x8_t.append(consts.tile([128, 2, T], E4M3, tag=f"x8{cc}", name=f"x8{cc}"))
        xr_t.append(consts.tile([128, 2, T], E4M3, tag=f"xr{cc}", name=f"xr{cc}"))
    for lo, hi in ((0, 512), (512, T)):
        for cc in range(CB2):
            nc.sync.dma_start(x8_t[cc][:, :, lo:hi], io["x8"].ap()[cc][:, :, lo:hi])
            nc.sync.dma_start(xr_t[cc][:, :, lo:hi], io["xr"].ap()[cc][:, :, lo:hi])

    bk_t, kt_t, y_t, wp_t, v_t = [], [], [], [], []
    for nb in range(NB):
        t = consts.tile([128, 1], F32, tag=f"bk{nb}", name=f"bk{nb}")
        nc.sync.dma_start(t[:], io["bk"].ap()[nb * 128:(nb + 1) * 128, :])
        bk_t.append(t)
        kt_t.append(consts.tile([128, T], F16, tag=f"kt{nb}", name=f"kt{nb}"))
        y_t.append(consts.tile([128, T], F16, tag=f"y{nb}", name=f"y{nb}"))
        t = consts.tile([128, COUT], F16, tag=f"wp{nb}", name=f"wp{nb}")
        nc.sync.dma_start(t[:], io["wp"].ap()[nb * 128:(nb + 1) * 128, :])
        wp_t.append(t)
    for tb in range(TB):
        # V tiles: per head-pair hp, two slots [V_h(64) | 1] of width 65
        t = consts.tile([128, NB, 2, 65], F16, tag=f"v{tb}", name=f"v{tb}")
        nc.gpsimd.memset(t[:, :, :, 64:65], 1.0)
        v_t.append(t)

    def kt_phase(hp):
        # KT[n,t] = ((x8+xr) @ (w8+wr)/WSCALE + bk)^T for this head pair
        for tch in range(NCH):
            ps = kpsum.tile([128, 512], F32, tag="kps", name="pskt")
            terms = [(w8_t, x8_t), (wr_t, x8_t), (w8_t, xr_t)]
            n_steps = len(terms) * CB2
            i = 0
            for wt, xt in terms:
                for cc in range(CB2):
                    nc.tensor.matmul(
                        ps[:],
                        wt[cc][:, :, hp * 128:(hp + 1) * 128],
                        xt[cc][:, :, tch * 512:(tch + 1) * 512],
                        start=(i == 0), stop=(i == n_steps - 1),
                        perf_mode=DR,
                    )
                    i += 1
            nc.vector.tensor_scalar(
                kt_t[hp][:, tch * 512:(tch + 1) * 512], ps[:],
                1.0 / WSCALE, bk_t[hp][:], op0=MUL, op1=ADD)

    def v_phase(hp):
        # V natural from PE transpose of KT; halves go to the two 65-slots
        for tb in range(TB):
            ps = kpsum.tile([128, 128], F16, tag="kps", name="pst")
            nc.tensor.transpose(ps[:], kt_t[hp][:, tb * 128:(tb + 1) * 128], id_t[:])
            nc.vector.tensor_copy(
                v_t[tb][:, hp, :, 0:64],
                ps[:].rearrange("p (h c) -> p h c", h=2))

    strip_ctr = [0]

    def s_strip(hp, ci, jb):
        # S^T strip [j-block jb, i in chunk ci], both heads; exp on ACT or Pool
        off = max(0, 128 * jb - 512 * ci)
        N = 512 - off
        ilo = 512 * ci + off
        ps = spsum.tile([128, 1024], F32, tag="sps", name="psS")
        nc.tensor.matmul(ps[:, 0:N],
                         kt_t[hp][0:64, jb * 128:(jb + 1) * 128],
                         kt_t[hp][0:64, ilo:ilo + N],
                         start=True, stop=True)
        nc.tensor.matmul(ps[:, 512:512 + N],
                         kt_t[hp][64:128, jb * 128:(jb + 1) * 128],
                         kt_t[hp][64:128, ilo:ilo + N],
                         start=True, stop=True)
        et = epool.tile([128, 1024], F16, tag="e", name="et")
        src = ps[:].rearrange("p (a c) -> p a c", a=2)[:, :, 0:N]
        dst = et[:].rearrange("p (a c) -> p a c", a=2)[:, :, 0:N]
        k = strip_ctr[0]
        strip_ctr[0] += 1
        if pool_exp_stride and k % pool_exp_stride == pool_exp_stride - 1:
            # DVE stages PSUM->SBUF, Pool does base**S (GPSIMD can't see PSUM)
            ss = epool.tile([128, 1024], F32, tag="ss", name="ss")
            sdst = ss[:].rearrange("p (a c) -> p a c", a=2)[:, :, 0:N]
            nc.vector.tensor_copy(sdst, src)
            nc.gpsimd.tensor_tensor(
                dst, base_t[:].rearrange("p (a c) -> p a c", a=2)[:, :, 0:N],
                sdst, POW)
        else:
            nc.scalar.activation(dst, src, EXP, scale=1.0 / 8.0)
        if jb >= 4 * ci:  # strip starts at the diagonal block: mask it
            nc.gpsimd.tensor_mul(et[:, 0:128], et[:, 0:128], tri_t[:])
            nc.gpsimd.tensor_mul(et[:, 512:640], et[:, 512:640], tri_t[:])
        return et

    def u_chain(hp, ci, ib, ue, uo, ets):
        # U natural for i-block 4*ci+ib: out[i, 0:64]=U, out[i,64]=denominator
        ib_g = 4 * ci + ib
        for jb in range(ib_g + 1):
            off = max(0, 128 * jb - 512 * ci)
            a = 128 * ib_g - (512 * ci + off)
            et = ets[jb]
            st, sp = (jb == 0), (jb == ib_g)
            nc.tensor.matmul(ue[:, ib, :], et[:, a:a + 128],
                             v_t[jb][:, hp, 0, :], start=st, stop=sp)
            nc.tensor.matmul(uo[:, ib, :], et[:, 512 + a:512 + a + 128],
                             v_t[jb][:, hp, 1, :], start=st, stop=sp)

    def norm_tr(hp, ci, ue, uo, with_proj):
        # normalize from PSUM, write yn natural, PE-transpose into y_t
        r = rpool.tile([128, 2, 4], F32, tag="r", name="r")
        nc.vector.reciprocal(
            r[:, 0, :], ue[:, :, 64:65].rearrange("p a one -> p (a one)"))
        nc.vector.reciprocal(
            r[:, 1, :], uo[:, :, 64:65].rearrange("p a one -> p (a one)"))
        yn = ynpool.tile([128, 4, 128], F16, tag="yn", name="yn")
        nc.vector.tensor_tensor(
            yn[:, :, 0:64], ue[:, :, 0:64],
            r[:, 0, :].to_broadcast([128, 4, 64]), MUL)
        nc.vector.tensor_tensor(
            yn[:, :, 64:128], uo[:, :, 0:64],
            r[:, 1, :].to_broadcast([128, 4, 64]), MUL)
        for ib in range(4):
            tb = 4 * ci + ib
            ps = kpsum.tile([128, 128], F16, tag="kps", name="psy")
            nc.tensor.transpose(ps[:], yn[:, ib, :], id_t[:])
            nc.vector.tensor_copy(y_t[hp][:, tb * 128:(tb + 1) * 128], ps[:])
            if with_proj:
                proj_tb(tb)

    def proj_tb(tb):
        ot = opool.tile([128, COUT], F16, tag="o", name="ot")
        for nh in range(COUT // 512):
            po = kpsum.tile([128, 512], F32, tag="kps", name="po")
            for hp2 in range(NB):
                nc.tensor.matmul(po[:],
                                 y_t[hp2][:, tb * 128:(tb + 1) * 128],
                                 wp_t[hp2][:, nh * 512:(nh + 1) * 512],
                                 start=(hp2 == 0), stop=(hp2 == NB - 1))
            nc.vector.tensor_copy(ot[:, nh * 512:(nh + 1) * 512], po[:])
        nc.sync.dma_start(io["out"].ap()[tb * 128:(tb + 1) * 128, :], ot[:])

    def att_phase(hp, with_proj):
        cis = list(range(NCH))
        if with_proj and desc_last:
            cis = cis[::-1]
        prev = None  # (ci, ets)
        for ci in cis:
            jmax = 4 * ci + 4
            ets = {}
            if prev is None:
                for jb in range(jmax):
                    ets[jb] = s_strip(hp, ci, jb)
            else:
                pci, pets = prev
                ue = upsum.tile([128, 4, 65], F32, tag="ue", name="ue")
                uo = upsum.tile([128, 4, 65], F32, tag="uo", name="uo")
                for k in range(max(jmax, 4)):
                    if k < jmax:
                        ets[k] = s_strip(hp, ci, k)
                    if k < 4:
                        u_chain(hp, pci, k, ue, uo, pets)
                norm_tr(hp, pci, ue, uo, with_proj)
            prev = (ci, ets)
        pci, pets = prev
        ue = upsum.tile([128, 4, 65], F32, tag="ue", name="ue")
        uo = upsum.tile([128, 4, 65], F32, tag="uo", name="uo")
        for ib in range(4):
            u_chain(hp, pci, ib, ue, uo, pets)
        norm_tr(hp, pci, ue, uo, with_proj)

    for hp in range(NB):
        kt_phase(hp)
        v_phase(hp)
        att_phase(hp, with_proj=(hp == NB - 1))


def make_inputs(cfg, x, Wk, bk, Wp):
    """Host-side input map for one core. x [T,CIN] fp32, Wk [CIN,NW], bk, Wp."""
    import numpy as np
    e4 = mybir.dt.np(E4M3)
    f16 = np.float16

    def pack(a, free):  # [CIN, free] -> [CB2, 128, 2, free]
        return np.ascontiguousarray(
            a.reshape(cfg.CB2, 2, 128, free).transpose(0, 2, 1, 3))

    xT = np.ascontiguousarray(x.T).astype(np.float32)
    x8 = xT.astype(e4)
    xr = (xT - x8.astype(np.float32)).astype(e4)
    ws = (Wk * WSCALE).astype(np.float32)
    w8 = ws.astype(e4)
    wr = (ws - w8.astype(np.float32)).astype(e4)

    jj, ii = np.meshgrid(np.arange(128), np.arange(128), indexing="ij")
    return {
        "x8": pack(x8, cfg.T), "xr": pack(xr, cfg.T),
        "w8": pack(w8, cfg.NW), "wr": pack(wr, cfg.NW),
        "bk": np.ascontiguousarray(bk.reshape(-1, 1)).astype(np.float32),
        "wp": np.ascontiguousarray(Wp).astype(f16),
        "tri": (jj <= ii).astype(f16),
        "ident": np.eye(128).astype(f16),
    }


# ======================================================================
# Host-side entry: shard across 8 NeuronCores as (batch x head-group),
# run the Bass kernel, gather + reduce partials on host.
# ======================================================================

import numpy as np

from concourse import bacc
from concourse.bass_utils import run_bass_kernel_spmd

B, T, C, H = 4, 2048, 1024, 16
N_CORES = 8
HG = 2                      # head groups (tensor-parallel axis)
NW = C // HG                # 512 columns of W_k per group

_cache = {}


def get_compiled():
    if "nc" not in _cache:
        cfg = Cfg(T=T, CIN=C, HL=H // HG, COUT=C)
        nc = bacc.Bacc("TRN2", target_bir_lowering=False, debug=False,
                       num_devices=N_CORES)
        io = declare_io(nc, cfg)
        with tile.TileContext(nc) as tc:
            with ExitStack() as ctx:
                build(ctx, tc, io, cfg)
        nc.compile()
        _cache["nc"] = (nc, cfg)
    return _cache["nc"]


def make_in_maps(cfg, x, W_attn, b_attn, W_proj):
    in_maps = []
    for core in range(N_CORES):
        b, hg = core // HG, core % HG
        sl = slice(C + hg * NW, C + (hg + 1) * NW)
        in_maps.append(make_inputs(
            cfg, x[b], W_attn[:, sl], b_attn[sl],
            W_proj[hg * NW:(hg + 1) * NW, :]))
    return in_maps


def kernel(x, W_attn, b_attn, W_proj, b_proj):
    x = np.asarray(x, dtype=np.float32)
    W_attn = np.asarray(W_attn, dtype=np.float32)
    b_attn = np.asarray(b_attn, dtype=np.float32)
    W_proj = np.asarray(W_proj, dtype=np.float32)
    b_proj = np.asarray(b_proj, dtype=np.float32)

    nc, cfg = get_compiled()
    in_maps = make_in_maps(cfg, x, W_attn, b_attn, W_proj)
    res = run_bass_kernel_spmd(nc, in_maps, core_ids=list(range(N_CORES)))
    out = np.empty((B, T, C), dtype=np.float32)
    for b in range(B):
        out[b] = (res.results[HG * b]["out"].astype(np.float32)
                  + res.results[HG * b + 1]["out"].astype(np.float32)
                  + b_proj[None, :])
    return out


# revision 16
# speedup vs baseline: 1.1652x; 1.1652x over previous
"""Per-core causal self-attention kernel (Bass/Tile, TRN2), v3.

One core handles one batch b and one head-group of HL=8 heads (q=k=v quirk:
only the k-slice of W_attn matters):
    K  = x @ Wk + bk                    # [T, NW], NW = HL*64
    per head h: S = K_h K_h^T / 8 (causal), P = exp (no max-sub)
    Y_h = (P @ K_h) / rowsum(P)
    out_partial = Y @ Wp                # [T, COUT]; host sums partials + b_proj

Design notes:
  * kt-gen via fp8e4 DoubleRow matmuls with residual correction:
    K ~= (x8 @ w8 + x8 @ wr + xr @ w8)/WSCALE, w quantized at x64 scale
    (12 DR steps at half rate vs 8 fp16 steps: 25% less PE, MORE accurate
    than bf16 since the residual captures x/W to ~0.1%).
  * U computed in NATURAL layout: stationary = E^T 128x128 block, moving =
    [V_h | ones] (65 cols) accumulated over j-blocks in PSUM. 65-col moving
    vs v1's 512-col strips: halves U's PE time. The ones column yields the
    softmax denominator in col 64. Y^T for the projection is recovered by
    PE transposes of the normalized natural chunk (two heads paired to fill
    128 partitions), landing in the same y layout v1 used.
  * kt-gen and V-transposes for chunk ci are folded INTO the attention loop
    and the NEXT head-pair's kt/V is prefetched during the current pair's
    attention, so ACT never drains at phase boundaries and the initial x
    DMAs (chunked per 512 columns) overlap the first chunks' compute.
  * U PSUM is allocated per half-chunk [128, 2ib, 2head, 65] (1 bank) with
    bufs=2, and normalize runs per half straight out of PSUM (strided
    reciprocal of the denominator columns + one broadcast mul), so U
    accumulation of chunk ci overlaps normalize of ci-1.
  * S strips of chunk ci interleave with U chains of chunk ci-1 on PE.
  * exp on ACT (scale=1/8 folded in); causal masks on Pool (GPSIMD may not
    touch PSUM, but the et strips live in SBUF).
"""

from contextlib import ExitStack

import concourse.bass as bass
import concourse.tile as tile
from concourse import mybir

F32 = mybir.dt.float32
F16 = mybir.dt.float16
BF16 = mybir.dt.bfloat16
E4M3 = mybir.dt.float8e4
EXP = mybir.ActivationFunctionType.Exp
DR = mybir.MatmulPerfMode.DoubleRow
MUL = mybir.AluOpType.mult
ADD = mybir.AluOpType.add
POW = mybir.AluOpType.pow

WSCALE = 64.0  # power-of-two pre-scale of Wk so fp8e4 residuals stay normal


class Cfg:
    def __init__(self, T=2048, CIN=1024, HL=8, COUT=1024):
        self.T, self.CIN, self.HL, self.COUT = T, CIN, HL, COUT
        assert HL % 2 == 0 and T % 512 == 0 and CIN % 256 == 0 and COUT % 512 == 0
        self.NW = HL * 64          # n width (local head dims)
        self.NB = self.NW // 128   # head-pair blocks
        self.TB = T // 128         # t blocks
        self.NCH = T // 512        # i chunks
        self.CB2 = CIN // 256      # fp8 DoubleRow c-pair chunks


def declare_io(nc, cfg):
    io = {}
    # fp8 DoubleRow operands, host-packed as [128, CB2, 2, free]
    io["x8"] = nc.dram_tensor("x8", [128, cfg.CB2, 2, cfg.T], E4M3, kind="ExternalInput")
    io["xr"] = nc.dram_tensor("xr", [128, cfg.CB2, 2, cfg.T], E4M3, kind="ExternalInput")
    io["w8"] = nc.dram_tensor("w8", [128, cfg.CB2, 2, cfg.NW], E4M3, kind="ExternalInput")
    io["wr"] = nc.dram_tensor("wr", [128, cfg.CB2, 2, cfg.NW], E4M3, kind="ExternalInput")
    io["bk"] = nc.dram_tensor("bk", [128, cfg.NB, 1], F32, kind="ExternalInput")
    io["wp"] = nc.dram_tensor("wp", [128, cfg.NB, cfg.COUT], F16, kind="ExternalInput")
    io["tri"] = nc.dram_tensor("tri", [128, 128], F16, kind="ExternalInput")
    io["ident"] = nc.dram_tensor("ident", [128, 128], F16, kind="ExternalInput")
    io["out"] = nc.dram_tensor("out", [cfg.T, cfg.COUT], F16, kind="ExternalOutput")
    return io


def build(ctx: ExitStack, tc: tile.TileContext, io, cfg: Cfg, pool_exp_stride=0):
    nc = tc.nc
    T, HL, NB, TB, NCH, CB2, COUT = (cfg.T, cfg.HL, cfg.NB, cfg.TB, cfg.NCH,
                                     cfg.CB2, cfg.COUT)

    consts = ctx.enter_context(tc.tile_pool(name="consts", bufs=1))
    # PSUM budget (8 banks): sps 2x[128,1024]=4, u 2x(1 bank)=2, kps 2x=2
    spsum = ctx.enter_context(tc.tile_pool(name="sps", bufs=2, space="PSUM"))
    upsum = ctx.enter_context(tc.tile_pool(name="ups", bufs=2, space="PSUM"))
    kpsum = ctx.enter_context(tc.tile_pool(name="kps", bufs=2, space="PSUM"))
    epool = ctx.enter_context(tc.tile_pool(name="e", bufs=30))
    ynpool = ctx.enter_context(tc.tile_pool(name="yn", bufs=4))
    rpool = ctx.enter_context(tc.tile_pool(name="r", bufs=4))
    opool = ctx.enter_context(tc.tile_pool(name="o", bufs=3))

    # ---- persistent SBUF tensors; DMA order is pipeline-critical ----
    w8_t = consts.tile([128, CB2, 2, cfg.NW], E4M3, tag="w8")
    nc.sync.dma_start(w8_t[:], io["w8"].ap())
    wr_t = consts.tile([128, CB2, 2, cfg.NW], E4M3, tag="wr")
    nc.sync.dma_start(wr_t[:], io["wr"].ap())
    tri_t = consts.tile([128, 128], F16, tag="tri")
    nc.sync.dma_start(tri_t[:], io["tri"].ap())
    id_t = consts.tile([128, 128], F16, tag="ident")
    nc.sync.dma_start(id_t[:], io["ident"].ap())
    bk_t = consts.tile([128, NB, 1], F32, tag="bk")
    nc.sync.dma_start(bk_t[:], io["bk"].ap())

    x8_t = consts.tile([128, CB2, 2, T], E4M3, tag="x8")
    xr_t = consts.tile([128, CB2, 2, T], E4M3, tag="xr")
    for tch in range(NCH):
        sl = slice(tch * 512, (tch + 1) * 512)
        nc.sync.dma_start(x8_t[:, :, :, sl], io["x8"].ap()[:, :, :, sl])
        nc.sync.dma_start(xr_t[:, :, :, sl], io["xr"].ap()[:, :, :, sl])

    wp_t = consts.tile([128, NB, COUT], F16, tag="wp")
    nc.sync.dma_start(wp_t[:], io["wp"].ap())

    base_t = consts.tile([128, 1024], F32, tag="base")
    nc.gpsimd.memset(base_t[:], 1.1331484530668263)  # e**(1/8)

    kt_t, y_t, v_t = [], [], []
    for nb in range(NB):
        kt_t.append(consts.tile([128, T], F16, tag=f"kt{nb}", name=f"kt{nb}"))
        y_t.append(consts.tile([128, T], F16, tag=f"y{nb}", name=f"y{nb}"))
    for tb in range(TB):
        # V tiles: per head-pair hp, two slots [V_h(64) | 1] of width 65
        t = consts.tile([128, NB, 2, 65], F16, tag=f"v{tb}", name=f"v{tb}")
        nc.gpsimd.memset(t[:, :, :, 64:65], 1.0)
        v_t.append(t)

    def kt_tch(hp, tch):
        # KT[n,t] = ((x8+xr) @ (w8+wr)/WSCALE + bk)^T, one 512-col chunk
        ps = kpsum.tile([128, 512], F32, tag="kps", name="pskt")
        terms = [(w8_t, x8_t), (wr_t, x8_t), (w8_t, xr_t)]
        n_steps = len(terms) * CB2
        i = 0
        for wt, xt in terms:
            for cc in range(CB2):
                nc.tensor.matmul(
                    ps[:],
                    wt[:, cc, :, hp * 128:(hp + 1) * 128],
                    xt[:, cc, :, tch * 512:(tch + 1) * 512],
                    start=(i == 0), stop=(i == n_steps - 1),
                    perf_mode=DR,
                )
                i += 1
        nc.vector.tensor_scalar(
            kt_t[hp][:, tch * 512:(tch + 1) * 512], ps[:],
            1.0 / WSCALE, bk_t[:, hp, :], op0=MUL, op1=ADD)

    def v_tb(hp, tb):
        # V natural from PE transpose of KT; halves go to the two 65-slots
        ps = kpsum.tile([128, 128], F16, tag="kps", name="pst")
        nc.tensor.transpose(ps[:], kt_t[hp][:, tb * 128:(tb + 1) * 128], id_t[:])
        nc.vector.tensor_copy(
            v_t[tb][:, hp, :, 0:64],
            ps[:].rearrange("p (h c) -> p h c", h=2))

    strip_ctr = [0]

    def s_strip(hp, ci, jb):
        # S^T strip [j-block jb, i in chunk ci], both heads; exp; mask diag
        off = max(0, 128 * jb - 512 * ci)
        N = 512 - off
        ilo = 512 * ci + off
        ps = spsum.tile([128, 1024], F32, tag="sps", name="psS")
        nc.tensor.matmul(ps[:, 0:N],
                         kt_t[hp][0:64, jb * 128:(jb + 1) * 128],
                         kt_t[hp][0:64, ilo:ilo + N],
                         start=True, stop=True)
        nc.tensor.matmul(ps[:, 512:512 + N],
                         kt_t[hp][64:128, jb * 128:(jb + 1) * 128],
                         kt_t[hp][64:128, ilo:ilo + N],
                         start=True, stop=True)
        et = epool.tile([128, 1024], F16, tag="e", name="et")
        src = ps[:].rearrange("p (a c) -> p a c", a=2)[:, :, 0:N]
        dst = et[:].rearrange("p (a c) -> p a c", a=2)[:, :, 0:N]
        k = strip_ctr[0]
        strip_ctr[0] += 1
        if pool_exp_stride and k % pool_exp_stride == pool_exp_stride - 1:
            # DVE stages PSUM->SBUF, Pool does base**S (GPSIMD can't see PSUM)
            ss = epool.tile([128, 1024], F32, tag="ss", name="ss")
            sdst = ss[:].rearrange("p (a c) -> p a c", a=2)[:, :, 0:N]
            nc.vector.tensor_copy(sdst, src)
            nc.gpsimd.tensor_tensor(
                dst, base_t[:].rearrange("p (a c) -> p a c", a=2)[:, :, 0:N],
                sdst, POW)
        else:
            nc.scalar.activation(dst, src, EXP, scale=1.0 / 8.0)
        if jb >= 4 * ci:  # strip starts at the diagonal block: mask it
            nc.gpsimd.tensor_mul(et[:, 0:128], et[:, 0:128], tri_t[:])
            nc.gpsimd.tensor_mul(et[:, 512:640], et[:, 512:640], tri_t[:])
        return et

    def u_half(hp, ci, h, ets):
        # U natural for i-blocks 4ci+2h, 4ci+2h+1: [128, ib2, head, 65]
        ut = upsum.tile([128, 2, 2, 65], F32, tag="u", name="ut")
        for ib2 in range(2):
            ib_g = 4 * ci + 2 * h + ib2
            # chains strictly sequential within the bank: each accumulation
            # group runs start..stop before the next one begins
            for head in range(2):
                for jb in range(ib_g + 1):
                    off = max(0, 128 * jb - 512 * ci)
                    a = 128 * ib_g - (512 * ci + off) + 512 * head
                    st, sp = (jb == 0), (jb == ib_g)
                    nc.tensor.matmul(ut[:, ib2, head, :], ets[jb][:, a:a + 128],
                                     v_t[jb][:, hp, head, :], start=st, stop=sp)
        return ut

    def norm_half(hp, ci, h, ut, with_proj):
        # normalize from PSUM, write yn natural, PE-transpose into y_t
        r = rpool.tile([128, 2, 2], F32, tag="r", name="r")
        nc.vector.reciprocal(
            r[:], ut[:, :, :, 64:65].rearrange("p a b one -> p a (b one)"))
        yn = ynpool.tile([128, 2, 2, 64], F16, tag="yn", name="yn")
        nc.vector.tensor_tensor(
            yn[:], ut[:, :, :, 0:64], r[:].to_broadcast([128, 2, 2, 64]), MUL)
        for ib2 in range(2):
            tb = 4 * ci + 2 * h + ib2
            ps = kpsum.tile([128, 128], F16, tag="kps", name="psy")
            nc.tensor.transpose(
                ps[:], yn[:, ib2, :, :].rearrange("p a b -> p (a b)"), id_t[:])
            nc.vector.tensor_copy(y_t[hp][:, tb * 128:(tb + 1) * 128], ps[:])
            if with_proj:
                proj_tb(tb)

    def proj_tb(tb):
        ot = opool.tile([128, COUT], F16, tag="o", name="ot")
        for nh in range(COUT // 512):
            po = kpsum.tile([128, 512], F32, tag="kps", name="po")
            for hp2 in range(NB):
                nc.tensor.matmul(po[:],
                                 y_t[hp2][:, tb * 128:(tb + 1) * 128],
                                 wp_t[:, hp2, nh * 512:(nh + 1) * 512],
                                 start=(hp2 == 0), stop=(hp2 == NB - 1))
            nc.vector.tensor_copy(ot[:, nh * 512:(nh + 1) * 512], po[:])
        nc.sync.dma_start(io["out"].ap()[tb * 128:(tb + 1) * 128, :], ot[:])

    def att_phase(hp, with_proj, asc, inline_kt, prefetch_hp):
        cis = list(range(NCH)) if asc else list(range(NCH))[::-1]
        pend = None  # (ci, ets)
        for idx, ci in enumerate(cis):
            if inline_kt:
                kt_tch(hp, ci)
                for tb in range(4 * ci, 4 * ci + 4):
                    v_tb(hp, tb)
            if prefetch_hp is not None:
                kt_tch(prefetch_hp, idx)
                for tb in range(4 * idx, 4 * idx + 4):
                    v_tb(prefetch_hp, tb)
            jmax = 4 * ci + 4
            ets = {}
            for j in range(jmax):
                ets[j] = s_strip(hp, ci, j)
                if pend is not None and j < 2:
                    pci, pets = pend
                    ut = u_half(hp, pci, j, pets)
                    norm_half(hp, pci, j, ut, with_proj)
            pend = (ci, ets)
        pci, pets = pend
        for h in range(2):
            ut = u_half(hp, pci, h, pets)
            norm_half(hp, pci, h, ut, with_proj)

    for hp in range(NB):
        att_phase(hp,
                  with_proj=(hp == NB - 1),
                  asc=(hp < NB - 1),
                  inline_kt=(hp == 0),
                  prefetch_hp=(hp + 1 if hp < NB - 1 else None))


def make_inputs(cfg, x, Wk, bk, Wp):
    """Host-side input map for one core. x [T,CIN] fp32, Wk [CIN,NW], bk, Wp."""
    import numpy as np
    e4 = mybir.dt.np(E4M3)
    f16 = np.float16

    def pack(a, free):  # [CIN, free] -> [128, CB2, 2, free]
        return np.ascontiguousarray(
            a.reshape(cfg.CB2, 2, 128, free).transpose(2, 0, 1, 3))

    xT = np.ascontiguousarray(x.T).astype(np.float32)
    x8 = xT.astype(e4)
    xr = (xT - x8.astype(np.float32)).astype(e4)
    ws = (Wk * WSCALE).astype(np.float32)
    w8 = ws.astype(e4)
    wr = (ws - w8.astype(np.float32)).astype(e4)

    jj, ii = np.meshgrid(np.arange(128), np.arange(128), indexing="ij")
    return {
        "x8": pack(x8, cfg.T), "xr": pack(xr, cfg.T),
        "w8": pack(w8, cfg.NW), "wr": pack(wr, cfg.NW),
        "bk": np.ascontiguousarray(
            bk.reshape(cfg.NB, 128, 1).transpose(1, 0, 2)).astype(np.float32),
        "wp": np.ascontiguousarray(
            Wp.reshape(cfg.NB, 128, cfg.COUT).transpose(1, 0, 2)).astype(f16),
        "tri": (jj <= ii).astype(f16),
        "ident": np.eye(128).astype(f16),
    }


# ======================================================================
# Host-side entry: shard across 8 NeuronCores as (batch x head-group),
# run the Bass kernel, gather + reduce partials on host.
# ======================================================================

import numpy as np

from concourse import bacc
from concourse.bass_utils import run_bass_kernel_spmd

B, T, C, H = 4, 2048, 1024, 16
N_CORES = 8
HG = 2                      # head groups (tensor-parallel axis)
NW = C // HG                # 512 columns of W_k per group

_cache = {}


def get_compiled():
    if "nc" not in _cache:
        cfg = Cfg(T=T, CIN=C, HL=H // HG, COUT=C)
        nc = bacc.Bacc("TRN2", target_bir_lowering=False, debug=False,
                       num_devices=N_CORES)
        io = declare_io(nc, cfg)
        with tile.TileContext(nc) as tc:
            with ExitStack() as ctx:
                build(ctx, tc, io, cfg)
        nc.compile()
        _cache["nc"] = (nc, cfg)
    return _cache["nc"]


def make_in_maps(cfg, x, W_attn, b_attn, W_proj):
    in_maps = []
    for core in range(N_CORES):
        b, hg = core // HG, core % HG
        sl = slice(C + hg * NW, C + (hg + 1) * NW)
        in_maps.append(make_inputs(
            cfg, x[b], W_attn[:, sl], b_attn[sl],
            W_proj[hg * NW:(hg + 1) * NW, :]))
    return in_maps


def kernel(x, W_attn, b_attn, W_proj, b_proj):
    x = np.asarray(x, dtype=np.float32)
    W_attn = np.asarray(W_attn, dtype=np.float32)
    b_attn = np.asarray(b_attn, dtype=np.float32)
    W_proj = np.asarray(W_proj, dtype=np.float32)
    b_proj = np.asarray(b_proj, dtype=np.float32)

    nc, cfg = get_compiled()
    in_maps = make_in_maps(cfg, x, W_attn, b_attn, W_proj)
    res = run_bass_kernel_spmd(nc, in_maps, core_ids=list(range(N_CORES)))
    out = np.empty((B, T, C), dtype=np.float32)
    for b in range(B):
        out[b] = (res.results[HG * b]["out"].astype(np.float32)
                  + res.results[HG * b + 1]["out"].astype(np.float32)
                  + b_proj[None, :])
    return out
